# revision 21
# baseline (speedup 1.0000x reference)
"""Trainium2 Bass kernel for the phase-type log-prior problem.

reference(w, S, alpha) = sum_m log( alpha^T expm(w_m * S) s ),  s = -S @ 1

Since S is a fixed matrix and w_m are scalars, expm(w_m S) = V diag(exp(w_m d)) V^-1
with (d, V) the eigendecomposition of S (real eigenvalues for the lower-triangular
phase-type sub-generator this problem uses).  Hence

    density[m] = sum_i c_i * exp(d_i * w_m),   c = (alpha^T V) * (V^-1 s)

The device kernel computes, per element: 8 exps (ScalarE activation with the
per-term scale d_i and bias ln|c_i| folded in), a sign-split strided reduction
over the 8 terms (VectorE), a log with per-partition accumulation (ScalarE),
yielding 128 partial sums per core.  Data parallel over 8 cores; the final
sum of 1024 partials happens on host in fp64.
"""

import os
import sys

import numpy as np

sys.path.insert(0, "/opt/trn_rl_repo")

import concourse.bass as bass  # noqa: E402
import concourse.mybir as mybir  # noqa: E402
import concourse.tile as tile  # noqa: E402
from concourse.bass_utils import run_bass_kernel_spmd  # noqa: E402

N_CORES = 8
F32 = mybir.dt.float32
AF = mybir.ActivationFunctionType

_program_cache: dict = {}
_last_results = None


def _build_program(d: np.ndarray, logc: np.ndarray, n_pos: int, P: int, J: int,
                   pad_front: int):
    """Build the SPMD Bass program.

    d, logc: per-term exp scale/bias, positives-first ordering.
    P, J: per-core tile layout [P partitions, J free]; shard size = P*J.
    pad_front: elements 0..pad_front-1 of the shard (= row 0, cols
               0..pad_front-1) are padding; their density is forced to 1 so
               they contribute log(1)=0.  (Padding sits at the front because
               SBUF instruction APs must start at a quadrant-aligned
               partition; row 0 always works.)
    """
    n_terms = len(d)
    n_neg = n_terms - n_pos

    nc = bass.Bass()
    # Single packed input: per partition row, J w-values then the n_terms
    # biases ln|c_i| (replicated on every row).  One DMA -> one semaphore,
    # because the TRN2 activation ISA slot supports only one sync wait.
    wb_in = nc.declare_dram_parameter("wb", [P, J + n_terms], F32,
                                      isOutput=False)
    out = nc.declare_dram_parameter("partials", [P, 1], F32, isOutput=True)

    with tile.TileContext(nc) as tc:
        with tc.tile_pool(name="main", bufs=1) as pool:
            # Tiny dummy activation first: walrus inserts the ~2.7us activation
            # table load before the first ACTIVATE; doing it on a 2-element tile
            # lets it overlap the input DMA.
            dum = pool.tile([1, 2], F32)
            nc.vector.memset(dum[:], 1.0)
            nc.scalar.activation(dum[:], dum[:], AF.Exp)

            WB = pool.tile([P, J + n_terms], F32)
            nc.sync.dma_start(WB[:], wb_in[:])
            W = WB[:, 0:J]
            B = WB[:, J:J + n_terms]

            # F[:, i*J:(i+1)*J] = |c_i| * exp(d_i * w)
            F = pool.tile([P, n_terms * J], F32)
            for i in range(n_terms):
                nc.scalar.activation(
                    F[:, i * J:(i + 1) * J], W, AF.Exp,
                    bias=B[:, i:i + 1], scale=float(d[i]),
                )

            dens = pool.tile([P, J], F32)
            if n_neg == 0:
                nc.vector.tensor_reduce(
                    dens[:], F[:].rearrange("p (i j) -> p j i", i=n_terms),
                    axis=mybir.AxisListType.X, op=mybir.AluOpType.add,
                )
            else:
                pos = pool.tile([P, J], F32)
                neg = pool.tile([P, J], F32)
                if n_pos == 1:
                    pos = F  # view: first block
                    pos_ap = F[:, 0:J]
                else:
                    nc.vector.tensor_reduce(
                        pos[:],
                        F[:, 0:n_pos * J].rearrange("p (i j) -> p j i", i=n_pos),
                        axis=mybir.AxisListType.X, op=mybir.AluOpType.add,
                    )
                    pos_ap = pos[:]
                if n_neg == 1:
                    neg_ap = F[:, n_pos * J:(n_pos + 1) * J]
                else:
                    nc.vector.tensor_reduce(
                        neg[:],
                        F[:, n_pos * J:].rearrange("p (i j) -> p j i", i=n_neg),
                        axis=mybir.AxisListType.X, op=mybir.AluOpType.add,
                    )
                    neg_ap = neg[:]
                nc.vector.tensor_sub(dens[:], pos_ap, neg_ap)

            # Padding prefix of partition row 0: force density=1 -> log=0.
            if pad_front > 0:
                nc.vector.memset(dens[0:1, 0:pad_front], 1.0)

            logd = pool.tile([P, J], F32)
            part = pool.tile([P, 1], F32)
            nc.scalar.activation(logd[:], dens[:], AF.Ln, accum_out=part[:])
            nc.sync.dma_start(out[:], part[:])

    _split_multiwait(nc)
    return nc


def _split_multiwait(nc, limit: int = 1):
    """walrus rejects instructions with more sync waits than the engine's ISA
    struct provides slots for (1 for Activation, ~3 for Drain).  Hoist excess
    waits into standalone NoOps on the same engine just before the
    instruction."""
    k = 0
    for fn in nc.m.functions:
        for blk in fn.blocks:
            new = []
            for inst in blk.instructions:
                si = getattr(inst, "sync_info", None)
                if si is not None and si.on_wait and len(si.on_wait) > limit:
                    waits = list(si.on_wait)
                    for wchunk in waits[:-limit]:
                        k += 1
                        new.append(mybir.InstNoOp(
                            name=f"wsplit-{k}-{inst.name}",
                            sync_info=mybir.SyncInfo(on_wait=[wchunk],
                                                     on_update=[]),
                            bass_nofuse=True,
                            engine=inst.engine,
                        ))
                    inst.sync_info = mybir.SyncInfo(on_wait=waits[-limit:],
                                                    on_update=si.on_update)
                new.append(inst)
            blk.instructions[:] = new


def _ensure_ntff_hook() -> bool:
    """The agent image lacks ``antenv.axon_hooks``; synthesize it and register
    the ctypes NTFF profile hook so trace=True works under axon."""
    try:
        from antenv.axon_hooks import get_axon_ntff_profile_hook
        return get_axon_ntff_profile_hook() is not None
    except ImportError:
        pass
    try:
        import types

        import antenv
        from trn_agent_boot.trn_boot import _ntff_profile_via_ctypes

        mod = types.ModuleType("antenv.axon_hooks")
        holder = {"hook": None}
        mod.set_axon_ntff_profile_hook = lambda h: holder.__setitem__("hook", h)
        mod.get_axon_ntff_profile_hook = lambda: holder["hook"]
        sys.modules["antenv.axon_hooks"] = mod
        antenv.axon_hooks = mod
        hook = _ntff_profile_via_ctypes("/opt/axon/libaxon_pjrt.so")
        if hook is None:
            return False
        mod.set_axon_ntff_profile_hook(hook)
        return True
    except Exception as e:  # pragma: no cover - profiling is best-effort
        print(f"NTFF hook setup failed: {e}", file=sys.stderr)
        return False


def _spectral_coeffs(S: np.ndarray, alpha: np.ndarray):
    """c_i, d_i with density(w) = sum_i c_i exp(d_i w)."""
    S64 = S.astype(np.float64)
    s_vec = -S64.sum(axis=1)
    d, V = np.linalg.eig(S64)
    c = (alpha.astype(np.float64) @ V) * np.linalg.solve(V, s_vec)
    if np.abs(d.imag).max() > 1e-8 or np.abs(c.imag).max() > 1e-6 * max(
            1.0, np.abs(c.real).max()):
        raise NotImplementedError("complex eigenvalues not supported")
    return c.real.copy(), d.real.copy()


def kernel(w: np.ndarray, S: np.ndarray, alpha: np.ndarray) -> np.ndarray:
    w = np.ascontiguousarray(np.asarray(w).reshape(-1), dtype=np.float32)
    S = np.asarray(S, dtype=np.float32)
    alpha = np.asarray(alpha, dtype=np.float32)

    c, d = _spectral_coeffs(S, alpha)
    # Drop numerically-zero terms, order positives first.
    keep = np.abs(c) > 1e-300
    c, d = c[keep], d[keep]
    order = np.argsort(c < 0, kind="stable")
    c, d = c[order], d[order]
    n_pos = int((c > 0).sum())
    logc = np.log(np.abs(c))

    M = w.size
    per = -(-M // N_CORES)          # ceil
    P = 128
    J = -(-per // P)                # ceil -> shard size P*J
    shard = P * J
    pad_front = shard - per         # leading pad elements per shard
    assert pad_front < J, "pad must fit in partition row 0"
    # Every core gets `per` real elements, padded at the FRONT to `shard`
    # with 1.0; the device masks the pad (density := 1 -> log contributes 0).
    assert M % N_CORES == 0, "expected evenly divisible batch"
    n_terms = d.size
    shards = []
    for i in range(N_CORES):
        sh = np.empty((P, J + n_terms), np.float32)
        wrow = np.empty(shard, np.float32)
        wrow[:pad_front] = 1.0
        wrow[pad_front:] = w[i * per:(i + 1) * per]
        sh[:, :J] = wrow.reshape(P, J)
        sh[:, J:] = logc.astype(np.float32)
        shards.append(sh)

    key = (d.tobytes(), logc.tobytes(), n_pos, P, J, pad_front)
    nc = _program_cache.get(key)
    if nc is None:
        nc = _build_program(d, logc, n_pos, P, J, pad_front)
        _program_cache[key] = nc

    in_maps = [{"wb": shards[i]} for i in range(N_CORES)]
    trace = bool(os.environ.get("KERNEL_TRACE"))
    if trace:
        trace = _ensure_ntff_hook()
    res = run_bass_kernel_spmd(nc, in_maps, list(range(N_CORES)), trace=trace)
    global _last_results
    _last_results = res
    total = 0.0
    for r in res.results:
        total += r["partials"].astype(np.float64).sum()
    return np.float32(total)


if __name__ == "__main__":
    z = np.load("/root/problem/inputs_cache.npz")
    out = kernel(z["w"], z["S"], z["alpha"])
    print("kernel output:", out)


# revision 23
# speedup vs baseline: 1.2996x; 1.2996x over previous
"""Trainium2 Bass kernel for the phase-type log-prior problem.

reference(w, S, alpha) = sum_m log( alpha^T expm(w_m * S) s ),  s = -S @ 1

Since S is a fixed matrix and w_m are scalars, expm(w_m S) = V diag(exp(w_m d)) V^-1
with (d, V) the eigendecomposition of S (real eigenvalues for the lower-triangular
phase-type sub-generator this problem uses).  Hence

    density[m] = sum_i c_i * exp(d_i * w_m),   c = (alpha^T V) * (V^-1 s)

Device kernel (per core, raw Bass for minimal overhead): one DMA of the
shard [128, J] (plus the 8 biases ln|c_i| packed in the same row), 8 Exp
activations with per-term scale/bias on ScalarE, an fp32 add-chain on
VectorE that trails the activations (positive terms first, then negative,
one subtract), then Ln with per-partition accumulation on ScalarE, and one
DMA of the 128 partial sums back out.  Data-parallel over 8 cores; host
sums the 1024 partials in fp64 and removes the padding contribution.
"""

import os
import sys

import numpy as np

sys.path.insert(0, "/opt/trn_rl_repo")

import concourse.bass as bass  # noqa: E402
import concourse.mybir as mybir  # noqa: E402
from concourse.bass_utils import run_bass_kernel_spmd  # noqa: E402

N_CORES = 8
F32 = mybir.dt.float32
AF = mybir.ActivationFunctionType

_program_cache: dict = {}
_last_results = None


def _build_program(d: np.ndarray, logc: np.ndarray, n_pos: int, P: int, J: int):
    """Raw-Bass SPMD program.

    d, logc: per-term exp scale / bias ln|c_i|, positives-first ordering.
    P, J: per-core tile layout [P partitions, J free]; shard size = P*J.
    Padding is handled on the host (pad value 1.0; its known log-density is
    subtracted from the total), so the device treats every element as real.
    """
    n_terms = len(d)
    n_neg = n_terms - n_pos
    assert n_pos >= 1

    nc = bass.Bass()
    wb_in = nc.declare_dram_parameter("wb", [P, J + n_terms], F32,
                                      isOutput=False)
    out = nc.declare_dram_parameter("partials", [P, 1], F32, isOutput=True)

    with (
        nc.sbuf_tensor([P, J + n_terms], F32) as WBt,
        nc.sbuf_tensor([P, n_terms * J], F32) as Ft,
        nc.sbuf_tensor([P, J], F32) as accp_t,
        nc.sbuf_tensor([P, J], F32) as accn_t,
        nc.sbuf_tensor([P, J], F32) as logd_t,
        nc.sbuf_tensor([P, 2], F32) as scratch_t,
        nc.semaphore("s_in") as s_in,
        nc.semaphore("s_act") as s_act,
        nc.semaphore("s_dve") as s_dve,
        nc.semaphore("s_out") as s_out,
        nc.Block() as block,
    ):
        WB = WBt[:]
        F = Ft[:]
        accp = accp_t[:]
        accn = accn_t[:]
        logd = logd_t[:]
        scratch = scratch_t[:]
        W = WB[:, 0:J]
        B = WB[:, J:J + n_terms]
        Fi = [F[:, i * J:(i + 1) * J] for i in range(n_terms)]
        part = scratch[:, 0:1]

        # Number of DVE ops (chain adds + final subtract):
        n_dve_ops = (n_pos - 1) + (0 if n_neg == 0 else (n_neg - 1) + 1)
        # Where the final density lives:
        if n_neg:
            dens = accn
        elif n_pos > 1:
            dens = accp
        else:
            dens = Fi[0]

        @block.sync
        def _(sync):
            sync.dma_start(WB, wb_in[:]).then_inc(s_in, 16)
            # n_acts = dummy + n_terms exps + Ln
            sync.wait_ge(s_act, n_terms + 2)
            sync.dma_start(out[:], part).then_inc(s_out, 16)
            sync.wait_ge(s_out, 16)

        @block.scalar
        def _(scalar):
            # Dummy activation: walrus hoists the ~1.5us activation table
            # load before it, overlapping the input DMA.  Reads the
            # const-1.0 AP so there is no data dependency.
            one = nc.const_aps.aps[(F32, 1.0)]
            nc.scalar.activation(scratch[0:1, 1:2], one[0:1], AF.Exp
                                 ).then_inc(s_act, 1)
            scalar.wait_ge(s_in, 16)
            for i in range(n_terms):
                nc.scalar.activation(
                    Fi[i], W, AF.Exp, bias=B[:, i:i + 1], scale=float(d[i]),
                ).then_inc(s_act, 1)
            if n_dve_ops:
                scalar.wait_ge(s_dve, n_dve_ops)
            nc.scalar.activation(logd, dens, AF.Ln, accum_out=part
                                 ).then_inc(s_act, 1)

        @block.vector
        def _(vector):
            # F_i is ready once s_act >= i+2 (dummy + i+1 exps).
            def emit_chain(acc, base, count):
                if count == 1:
                    return Fi[base]  # no op needed; caller waits as required
                vector.wait_ge(s_act, base + 1 + 2)
                nc.vector.tensor_add(acc, Fi[base], Fi[base + 1]
                                     ).then_inc(s_dve, 1)
                for k in range(2, count):
                    vector.wait_ge(s_act, base + k + 2)
                    nc.vector.tensor_add(acc, acc, Fi[base + k]
                                         ).then_inc(s_dve, 1)
                return acc

            pos = emit_chain(accp, 0, n_pos)
            if n_neg:
                neg = emit_chain(accn, n_pos, n_neg)
                # Ensure single-term operands are ready before the subtract
                # (the chains' own waits cover every other case).
                vector.wait_ge(s_act, n_terms + 1)
                nc.vector.tensor_sub(accn, pos, neg).then_inc(s_dve, 1)

    _split_multiwait(nc)
    return nc


def _split_multiwait(nc, limit: int = 1):
    """walrus rejects instructions whose embedded sync-wait list exceeds the
    engine ISA struct's slots (1 for Activation, ~3 for Drain).  Hoist excess
    waits into standalone NoOps on the same engine just before the
    instruction."""
    k = 0
    for fn in nc.m.functions:
        for blk in fn.blocks:
            new = []
            for inst in blk.instructions:
                si = getattr(inst, "sync_info", None)
                if si is not None and si.on_wait and len(si.on_wait) > limit:
                    waits = list(si.on_wait)
                    for wchunk in waits[:-limit]:
                        k += 1
                        new.append(mybir.InstNoOp(
                            name=f"wsplit-{k}-{inst.name}",
                            sync_info=mybir.SyncInfo(on_wait=[wchunk],
                                                     on_update=[]),
                            bass_nofuse=True,
                            engine=inst.engine,
                        ))
                    inst.sync_info = mybir.SyncInfo(on_wait=waits[-limit:],
                                                    on_update=si.on_update)
                new.append(inst)
            blk.instructions[:] = new


def _ensure_ntff_hook() -> bool:
    """The agent image lacks ``antenv.axon_hooks``; synthesize it and register
    the ctypes NTFF profile hook so trace=True works under axon."""
    try:
        from antenv.axon_hooks import get_axon_ntff_profile_hook
        return get_axon_ntff_profile_hook() is not None
    except ImportError:
        pass
    try:
        import types

        import antenv
        from trn_agent_boot.trn_boot import _ntff_profile_via_ctypes

        mod = types.ModuleType("antenv.axon_hooks")
        holder = {"hook": None}
        mod.set_axon_ntff_profile_hook = lambda h: holder.__setitem__("hook", h)
        mod.get_axon_ntff_profile_hook = lambda: holder["hook"]
        sys.modules["antenv.axon_hooks"] = mod
        antenv.axon_hooks = mod
        hook = _ntff_profile_via_ctypes("/opt/axon/libaxon_pjrt.so")
        if hook is None:
            return False
        mod.set_axon_ntff_profile_hook(hook)
        return True
    except Exception as e:  # pragma: no cover - profiling is best-effort
        print(f"NTFF hook setup failed: {e}", file=sys.stderr)
        return False


def _spectral_coeffs(S: np.ndarray, alpha: np.ndarray):
    """c_i, d_i with density(w) = sum_i c_i exp(d_i w)."""
    S64 = S.astype(np.float64)
    s_vec = -S64.sum(axis=1)
    d, V = np.linalg.eig(S64)
    c = (alpha.astype(np.float64) @ V) * np.linalg.solve(V, s_vec)
    if np.abs(d.imag).max() > 1e-8 or np.abs(c.imag).max() > 1e-6 * max(
            1.0, np.abs(c.real).max()):
        raise NotImplementedError("complex eigenvalues not supported")
    return c.real.copy(), d.real.copy()


def kernel(w: np.ndarray, S: np.ndarray, alpha: np.ndarray) -> np.ndarray:
    w = np.ascontiguousarray(np.asarray(w).reshape(-1), dtype=np.float32)
    S = np.asarray(S, dtype=np.float32)
    alpha = np.asarray(alpha, dtype=np.float32)

    c, d = _spectral_coeffs(S, alpha)
    # Drop numerically-zero terms, order positives first.
    keep = np.abs(c) > 1e-300
    c, d = c[keep], d[keep]
    order = np.argsort(c < 0, kind="stable")
    c, d = c[order], d[order]
    n_pos = int((c > 0).sum())
    logc = np.log(np.abs(c))

    M = w.size
    per = -(-M // N_CORES)          # ceil
    P = 128
    J = -(-per // P)                # ceil -> shard size P*J
    shard = P * J
    pad_per_core = shard - per
    assert M % N_CORES == 0, "expected evenly divisible batch"
    PAD_VAL = 1.0
    n_terms = d.size
    shards = []
    for i in range(N_CORES):
        sh = np.empty((P, J + n_terms), np.float32)
        wrow = np.empty(shard, np.float32)
        wrow[:per] = w[i * per:(i + 1) * per]
        wrow[per:] = PAD_VAL
        sh[:, :J] = wrow.reshape(P, J)
        sh[:, J:] = logc.astype(np.float32)
        shards.append(sh)

    key = (d.tobytes(), logc.tobytes(), n_pos, P, J)
    nc = _program_cache.get(key)
    if nc is None:
        nc = _build_program(d, logc, n_pos, P, J)
        _program_cache[key] = nc

    in_maps = [{"wb": shards[i]} for i in range(N_CORES)]
    trace = bool(os.environ.get("KERNEL_TRACE"))
    if trace:
        trace = _ensure_ntff_hook()
    res = run_bass_kernel_spmd(nc, in_maps, list(range(N_CORES)), trace=trace)
    global _last_results
    _last_results = res
    total = 0.0
    for r in res.results:
        total += r["partials"].astype(np.float64).sum()
    # Remove the host-known padding contribution log(density(PAD_VAL)).
    n_pad_total = N_CORES * pad_per_core
    if n_pad_total:
        total -= n_pad_total * float(np.log(np.exp(d * PAD_VAL) @ c))
    return np.float32(total)


if __name__ == "__main__":
    z = np.load("/root/problem/inputs_cache.npz")
    out = kernel(z["w"], z["S"], z["alpha"])
    print("kernel output:", out)
